# revision 28
# baseline (speedup 1.0000x reference)
"""HAN layer (4 metapaths x 2-layer mean-RGCN + metapath attention) on 8 trn2 cores.

Sharding: cores (2i, 2i+1) handle metapath i. Within a pair, L1 splits dst into
halves [0,nreg)/[nreg,2*nreg); after an in-pair AllGather of x1, L2 splits the
NREG range into quarters. Attention: score AllGather + ReduceScatter over the 4
cores holding the same node range ({0,2,4,6} and {1,3,5,7}).

Device algorithm per layer (linearity: segment_sum(x[src]) @ Wm): edges are
host-sorted by dst into groups of 128 dsts; an indirect DMA gathers x[src] rows
for a group; per 128-edge chunk a selector eq[e,d] = (dl[e]==d)*rec[e] is built
on DVE and matmul-accumulated on PE into meanT = (segment_mean)^T in PSUM; two
dense matmuls + fused ReLU produce the group's 128 output rows, written
contiguously (no scatter anywhere).

Host->device transfer dominates the end-to-end time (narrow tunnel link with a
large per-buffer fixed cost), so inputs are shipped compressed and
consolidated into 6 buffers: E int8-quantized (dequant scale folded into the
layer-1 weights) and sharded 1/8 per core (device AllGather rebuilds the full
table; layer-1 gather indices are composed as eids[src] so the per-metapath x0
never ships), edge grids at 4 bytes/slot (u16 idx-lo + u8 [idx-hi<<6|deg] + u8
dst-local), unpacked on the DVE with is_ge/subtract (no mod). All device
compute is bf16 with f32 PSUM accumulation.
"""

import math
import numpy as np

try:
    # run_bass_kernel_spmd re-jits an identical XLA wrapper on every call;
    # the persistent compilation cache makes those re-jits near-free.
    import jax as _jax
    _jax.config.update("jax_compilation_cache_dir", "/tmp/jax_cc")
    _jax.config.update("jax_persistent_cache_min_entry_size_bytes", -1)
    _jax.config.update("jax_persistent_cache_min_compile_time_secs", 0.0)
except Exception:
    pass

import concourse.bass as bass
import concourse.bacc as bacc
import concourse.mybir as mybir
from concourse.tile import TileContext
from concourse.bass_utils import run_bass_kernel_spmd

F32 = mybir.dt.float32
BF16 = mybir.dt.bfloat16
I32 = mybir.dt.int32
I8 = mybir.dt.int8
U16 = mybir.dt.uint16
U8 = mybir.dt.uint8

N_CORES = 8
BF = 4     # output groups batched per store DMA
CH = 16    # groups per grid-load DMA


def _np_bf16():
    import ml_dtypes
    return ml_dtypes.bfloat16


# ----------------------------------------------------------------- host prep

def _build_grids(idxs, dsts, lo, ng, nb, deg):
    """Packed grid: grid[p, g*nb + b] = edge at (partition p, chunk b) of group
    g; the indirect-DMA flat order j = p*nb + b lands row j at out-partition p,
    column block b. Ships as u16 idx-low + u8 (idx-high<<6 | deg) + u8
    dst-local. Empty slots: dl=128 (selector row all-zero), pk=1 (finite
    reciprocal)."""
    g = (dsts - lo) >> 7
    starts = np.searchsorted(dsts, lo + 128 * np.arange(ng))
    slot = np.arange(len(dsts)) - starts[g]
    p = slot & 127
    b = slot >> 7
    col = g * nb + b
    S = nb * ng
    degv = deg[dsts]
    assert degv.size == 0 or degv.max() <= 63
    assert idxs.size == 0 or idxs.max() < (1 << 18)
    lo16 = np.zeros((128, S), np.uint16)
    pk8 = np.ones((128, S), np.uint8)
    dl8 = np.full((128, S), 128, np.uint8)
    lo16[p, col] = (idxs & 0xFFFF).astype(np.uint16)
    pk8[p, col] = (((idxs >> 16) << 6) | degv).astype(np.uint8)
    dl8[p, col] = (dsts - lo - (g << 7)).astype(np.uint8)
    return lo16, pk8, dl8


def _group_max(dsts, lo, ng):
    starts = np.searchsorted(dsts, lo + 128 * np.arange(ng + 1))
    return int(np.diff(starts).max()) if len(dsts) else 1


# ------------------------------------------------------------- device build

def _emit_layer(nc, tc, pools, table, table_i8, n_hi, glo, lo_off, gu8,
                pk_off, dl_off, gidxd, xd_off, wm_t, wr_t, ng, nb, iota_t,
                ident_t, out_dram, rows_total, hook=None):
    sb, sbg, psum, sbeq = pools
    stage = None
    for g in range(ng):
        if g % CH == 0:
            w = min(CH, ng - g)
            lot = sbg.tile([128, nb * w], U16, tag="lot")
            nc.sync.dma_start(
                out=lot[:], in_=glo[:, lo_off + g * nb:lo_off + (g + w) * nb])
            pkt = sbg.tile([128, nb * w], U8, tag="pkt")
            nc.sync.dma_start(
                out=pkt[:], in_=gu8[:, pk_off + g * nb:pk_off + (g + w) * nb])
            dlt8 = sbg.tile([128, nb * w], U8, tag="dlt8")
            nc.sync.dma_start(
                out=dlt8[:], in_=gu8[:, dl_off + g * nb:dl_off + (g + w) * nb])
            idxdt = sbg.tile([128, w], I32, tag="idxdt")
            nc.sync.dma_start(out=idxdt[:],
                              in_=gidxd[:, xd_off + g:xd_off + g + w])
            # unpack pk = hi<<6 | deg without mod: 64*hi via is_ge steps,
            # idx = lo + 65536*hi (exact in f32: < 2^24), rec = 1/deg
            pkf = sbg.tile([128, nb * w], F32, tag="pkf")
            nc.vector.tensor_copy(out=pkf[:], in_=pkt[:])
            hi64 = sbg.tile([128, nb * w], F32, tag="hi64")
            nc.vector.tensor_scalar(out=hi64[:], in0=pkf[:], scalar1=64.0,
                                    scalar2=64.0, op0=mybir.AluOpType.is_ge,
                                    op1=mybir.AluOpType.mult)
            for k in range(1, n_hi):
                hpart = sbg.tile([128, nb * w], F32, tag="hpart")
                nc.vector.tensor_scalar(
                    out=hpart[:], in0=pkf[:], scalar1=64.0 * (k + 1),
                    scalar2=64.0, op0=mybir.AluOpType.is_ge,
                    op1=mybir.AluOpType.mult)
                nc.vector.tensor_tensor(out=hi64[:], in0=hi64[:], in1=hpart[:],
                                        op=mybir.AluOpType.add)
            dgf = sbg.tile([128, nb * w], F32, tag="dgf")
            nc.vector.tensor_tensor(out=dgf[:], in0=pkf[:], in1=hi64[:],
                                    op=mybir.AluOpType.subtract)
            lof = sbg.tile([128, nb * w], F32, tag="lof")
            nc.vector.tensor_copy(out=lof[:], in_=lot[:])
            nc.vector.tensor_scalar(out=hi64[:], in0=hi64[:], scalar1=1024.0,
                                    scalar2=None, op0=mybir.AluOpType.mult)
            idxt = sbg.tile([128, nb * w], I32, tag="idxt")
            nc.vector.tensor_tensor(out=idxt[:], in0=hi64[:], in1=lof[:],
                                    op=mybir.AluOpType.add)
            dlf = sbg.tile([128, nb * w], F32, tag="dlf")
            nc.vector.tensor_copy(out=dlf[:], in_=dlt8[:])
            recf = sbg.tile([128, nb * w], F32, tag="recf")
            nc.vector.reciprocal(out=recf[:], in_=dgf[:])
        o = (g % CH) * nb

        if table_i8:
            msgs_raw = sb.tile([128, nb * 128], I8, tag="msgs_raw")
        else:
            msgs_raw = sb.tile([128, nb * 128], BF16, tag="msgs")
        for b in range(nb):
            nc.gpsimd.indirect_dma_start(
                out=msgs_raw[:, b * 128:(b + 1) * 128], out_offset=None,
                in_=table[:],
                in_offset=bass.IndirectOffsetOnAxis(
                    ap=idxt[:, o + b:o + b + 1], axis=0))
        if table_i8:
            msgs = sb.tile([128, nb * 128], BF16, tag="msgs")
            nc.vector.tensor_copy(out=msgs[:], in_=msgs_raw[:])
        else:
            msgs = msgs_raw

        meant_ps = psum.tile([128, 128], F32, space="PSUM", tag="meant")
        for b in range(nb):
            eq = sbeq.tile([128, 128], BF16, tag="eq")
            nc.vector.tensor_scalar(
                out=eq[:], in0=iota_t[:],
                scalar1=dlf[:, o + b:o + b + 1], scalar2=recf[:, o + b:o + b + 1],
                op0=mybir.AluOpType.is_equal, op1=mybir.AluOpType.mult)
            nc.tensor.matmul(out=meant_ps[:], lhsT=msgs[:, b * 128:(b + 1) * 128],
                             rhs=eq[:], start=(b == 0), stop=(b == nb - 1))
        meant = sb.tile([128, 128], BF16, tag="meant_sb")
        nc.vector.tensor_copy(out=meant[:], in_=meant_ps[:])

        if table_i8:
            xd_raw = sb.tile([128, 128], I8, tag="xd_raw")
        else:
            xd_raw = sb.tile([128, 128], BF16, tag="xd")
        nc.gpsimd.indirect_dma_start(
            out=xd_raw[:], out_offset=None, in_=table[:],
            in_offset=bass.IndirectOffsetOnAxis(
                ap=idxdt[:, g % CH:g % CH + 1], axis=0))
        if table_i8:
            xd = sb.tile([128, 128], BF16, tag="xd")
            nc.vector.tensor_copy(out=xd[:], in_=xd_raw[:])
        else:
            xd = xd_raw
        xdt_ps = psum.tile([128, 128], BF16, space="PSUM", tag="xdt")
        nc.tensor.transpose(out=xdt_ps[:], in_=xd[:], identity=ident_t[:])
        xdt = sb.tile([128, 128], BF16, tag="xdt_sb")
        nc.vector.tensor_copy(out=xdt[:], in_=xdt_ps[:])

        h_ps = psum.tile([128, 128], F32, space="PSUM", tag="hps")
        nc.tensor.matmul(out=h_ps[:], lhsT=meant[:], rhs=wm_t[:],
                         start=True, stop=False)
        nc.tensor.matmul(out=h_ps[:], lhsT=xdt[:], rhs=wr_t[:],
                         start=False, stop=True)

        gb = g % BF
        if gb == 0:
            bw = min(BF, ng - g)
            stage = sb.tile([128, bw * 128], BF16, tag="xn_stage")
        xn = stage[:, gb * 128:(gb + 1) * 128]
        nc.scalar.activation(out=xn, in_=h_ps[:],
                             func=mybir.ActivationFunctionType.Relu)
        if hook is not None:
            hook(g, xn)
        if gb == bw - 1:
            g0 = g - gb
            rows = min((gb + 1) * 128, rows_total - g0 * 128)
            nfull = rows // 128
            if nfull > 0:
                nc.sync.dma_start(
                    out=out_dram[g0 * 128:g0 * 128 + nfull * 128, :]
                    .rearrange("(a t) f -> t a f", t=128),
                    in_=stage[:, :nfull * 128]
                    .rearrange("p (a f) -> p a f", f=128))
            rem = rows - nfull * 128
            if rem > 0:
                nc.sync.dma_start(
                    out=out_dram[g0 * 128 + nfull * 128:
                                 g0 * 128 + nfull * 128 + rem, :],
                    in_=stage[:rem, nfull * 128:(nfull + 1) * 128])


def build_program(n, nreg, etab, ng1, nb1, ng2, nb2):
    nc = bacc.Bacc("TRN2", target_bir_lowering=False, debug=False,
                   num_devices=N_CORES)
    half = nreg
    nsh = etab // N_CORES
    nrs = (ng2 * 128) // 4  # ReduceScatter rows per rank
    S1, S2 = nb1 * ng1, nb2 * ng2

    def ei(name, shape, dt=F32):
        return nc.dram_tensor(name, shape, dt, kind="ExternalInput")

    # consolidated inputs (per-buffer transfer overhead is large)
    e_sh = ei("e_sh", [nsh, 128], I8)
    g_lo = ei("g_lo", [128, S1 + S2], U16)          # [g1_lo | g2_lo]
    g_u8 = ei("g_u8", [128, 2 * (S1 + S2)], U8)     # [g1_pk|g1_dl|g2_pk|g2_dl]
    g_xd = ei("g_xd", [128, ng1 + ng2], I32)        # [idxd1 | idxd2]
    cst = ei("cst", [128, 132])                     # [iota | sel]
    wts = ei("wts", [128, 6 * 128], BF16)  # [wm1|wr1|wm2|wr2|qs|ident]

    out_part = nc.dram_tensor("out_part", [nrs, 128], BF16,
                              kind="ExternalOutput")

    e_loc = nc.dram_tensor("e_loc", [nsh, 128], I8)
    e_full = nc.dram_tensor("e_full", [nsh * N_CORES, 128], I8)
    x1_half = nc.dram_tensor("x1_half", [half, 128], BF16)
    x1_full = nc.dram_tensor("x1_full", [n, 128], BF16)
    x2b = nc.dram_tensor("x2b", [ng2 * 128, 128], BF16)
    sc_in = nc.dram_tensor("sc_in", [ng2, 128], F32)
    sc_all = nc.dram_tensor("sc_all", [4 * ng2, 128], F32)
    rs_in = nc.dram_tensor("rs_in", [ng2 * 128, 128], BF16)
    rs_out = nc.dram_tensor("rs_out", [nrs, 128], BF16)

    all_group = [list(range(N_CORES))]
    pair_groups = [[2 * i, 2 * i + 1] for i in range(4)]
    attn_groups = [[0, 2, 4, 6], [1, 3, 5, 7]]

    with TileContext(nc) as tc:
        with (
            tc.tile_pool(name="const", bufs=1) as cpool,
            tc.tile_pool(name="sb", bufs=3) as sb,
            tc.tile_pool(name="sbg", bufs=2) as sbg,
            tc.tile_pool(name="sbeq", bufs=4) as sbeq,
            tc.tile_pool(name="psum", bufs=2, space="PSUM") as psum,
        ):
            def cload(src, c0, cols, tag, dt):
                t = cpool.tile([128, cols], dt, tag=tag)
                nc.sync.dma_start(out=t[:], in_=src[:, c0:c0 + cols])
                return t

            iota_t = cload(cst, 0, 128, "c_iota", F32)
            sel_t = cload(cst, 128, 4, "c_sel", F32)
            wm1_t = cload(wts, 0, 128, "c_wm1", BF16)
            wr1_t = cload(wts, 128, 128, "c_wr1", BF16)
            wm2_t = cload(wts, 256, 128, "c_wm2", BF16)
            wr2_t = cload(wts, 384, 128, "c_wr2", BF16)
            qs_t = cload(wts, 512, 128, "c_qs", BF16)
            ident_t = cload(wts, 640, 128, "c_ident", BF16)
            score_sb = cpool.tile([128, ng2], F32, tag="c_score")

            pools = (sb, sbg, psum, sbeq)

            nc.sync.dma_start(out=e_loc[:, :], in_=e_sh[:, :])
            nc.gpsimd.collective_compute(
                "AllGather", mybir.AluOpType.bypass,
                replica_groups=all_group,
                ins=[e_loc[:, :]], outs=[e_full[:, :]])

            _emit_layer(nc, tc, pools, e_full, True, 3, g_lo, 0, g_u8, 0, S1,
                        g_xd, 0, wm1_t, wr1_t, ng1, nb1, iota_t, ident_t,
                        x1_half, half)

            nc.gpsimd.collective_compute(
                "AllGather", mybir.AluOpType.bypass,
                replica_groups=pair_groups,
                ins=[x1_half[:, :]], outs=[x1_full[:, :]])

            def score_hook(g, xn):
                t = sb.tile([128, 128], F32, tag="sc_tmp")
                nc.vector.tensor_tensor(out=t[:], in0=xn, in1=qs_t[:],
                                        op=mybir.AluOpType.mult)
                nc.vector.reduce_sum(out=score_sb[:, g:g + 1], in_=t[:],
                                     axis=mybir.AxisListType.X)

            _emit_layer(nc, tc, pools, x1_full, False, 1, g_lo, S1, g_u8,
                        2 * S1, 2 * S1 + S2, g_xd, ng1, wm2_t, wr2_t, ng2, nb2,
                        iota_t, ident_t, x2b, ng2 * 128, hook=score_hook)

            nc.sync.dma_start(out=sc_in[:, :].rearrange("t p -> p t"),
                              in_=score_sb[:, :])
            nc.gpsimd.collective_compute(
                "AllGather", mybir.AluOpType.bypass,
                replica_groups=attn_groups,
                ins=[sc_in[:, :]], outs=[sc_all[:, :]])

            # softmax over 4 metapaths (elementwise across four [128,ng2] tiles)
            s_t = []
            for p in range(4):
                st = cpool.tile([128, ng2], F32, tag=f"s{p}")
                nc.sync.dma_start(
                    out=st[:],
                    in_=sc_all[p * ng2:(p + 1) * ng2, :].rearrange("t p -> p t"))
                s_t.append(st)
            m = cpool.tile([128, ng2], F32, tag="c_m")
            nc.vector.tensor_tensor(out=m[:], in0=s_t[0][:], in1=s_t[1][:],
                                    op=mybir.AluOpType.max)
            for p in (2, 3):
                nc.vector.tensor_tensor(out=m[:], in0=m[:], in1=s_t[p][:],
                                        op=mybir.AluOpType.max)
            e_t = []
            for p in range(4):
                dt_ = cpool.tile([128, ng2], F32, tag=f"d{p}")
                nc.vector.tensor_tensor(out=dt_[:], in0=s_t[p][:], in1=m[:],
                                        op=mybir.AluOpType.subtract)
                et = cpool.tile([128, ng2], F32, tag=f"e{p}")
                nc.scalar.activation(out=et[:], in_=dt_[:],
                                     func=mybir.ActivationFunctionType.Exp)
                e_t.append(et)
            z = cpool.tile([128, ng2], F32, tag="c_z")
            nc.vector.tensor_tensor(out=z[:], in0=e_t[0][:], in1=e_t[1][:],
                                    op=mybir.AluOpType.add)
            for p in (2, 3):
                nc.vector.tensor_tensor(out=z[:], in0=z[:], in1=e_t[p][:],
                                        op=mybir.AluOpType.add)
            rz = cpool.tile([128, ng2], F32, tag="c_rz")
            nc.vector.reciprocal(out=rz[:], in_=z[:])
            wown = cpool.tile([128, ng2], F32, tag="c_wown")
            acc = cpool.tile([128, ng2], F32, tag="c_acc")
            nc.vector.tensor_scalar(out=wown[:], in0=e_t[0][:],
                                    scalar1=sel_t[:, 0:1], scalar2=None,
                                    op0=mybir.AluOpType.mult)
            for p in (1, 2, 3):
                nc.vector.tensor_scalar(out=acc[:], in0=e_t[p][:],
                                        scalar1=sel_t[:, p:p + 1], scalar2=None,
                                        op0=mybir.AluOpType.mult)
                nc.vector.tensor_tensor(out=wown[:], in0=wown[:], in1=acc[:],
                                        op=mybir.AluOpType.add)
            nc.vector.tensor_tensor(out=wown[:], in0=wown[:], in1=rz[:],
                                    op=mybir.AluOpType.mult)

            # weighted partials, batched BF groups per DMA
            for g0 in range(0, ng2, BF):
                bw = min(BF, ng2 - g0)
                xt = sb.tile([128, bw * 128], BF16, tag="attn_x")
                nc.sync.dma_start(
                    out=xt[:].rearrange("p (a f) -> p a f", f=128),
                    in_=x2b[g0 * 128:(g0 + bw) * 128, :]
                    .rearrange("(a t) f -> t a f", t=128))
                wt = sb.tile([128, bw * 128], BF16, tag="attn_w")
                for j in range(bw):
                    nc.vector.tensor_scalar(
                        out=wt[:, j * 128:(j + 1) * 128],
                        in0=xt[:, j * 128:(j + 1) * 128],
                        scalar1=wown[:, g0 + j:g0 + j + 1], scalar2=None,
                        op0=mybir.AluOpType.mult)
                nc.sync.dma_start(
                    out=rs_in[g0 * 128:(g0 + bw) * 128, :]
                    .rearrange("(a t) f -> t a f", t=128),
                    in_=wt[:].rearrange("p (a f) -> p a f", f=128))

            nc.gpsimd.collective_compute(
                "ReduceScatter", mybir.AluOpType.add,
                replica_groups=attn_groups,
                ins=[rs_in[:, :]], outs=[rs_out[:, :]])

            # rs_out [nrs,128] -> out_part, bounced through SBUF
            nblk = nrs // 128
            fin = cpool.tile([128, nblk * 128], BF16, tag="c_fin")
            nc.sync.dma_start(
                out=fin[:].rearrange("p (a f) -> p a f", f=128),
                in_=rs_out[:, :].rearrange("(a t) f -> t a f", t=128))
            nc.sync.dma_start(
                out=out_part[:, :].rearrange("(a t) f -> t a f", t=128),
                in_=fin[:].rearrange("p (a f) -> p a f", f=128))
    return nc


# ----------------------------------------------------------------- kernel()

def kernel(E, metapath_emb, W_root, W_rel, b, Wq, bq, edge_index, eids,
           nreg=50000, trace=False, debug=False):
    bf16 = _np_bf16()
    P = edge_index.shape[0]
    n = eids.shape[1]
    d = E.shape[1]
    etab = E.shape[0]
    scale = np.float32(1.0 / math.sqrt(d))
    assert P == 4 and d == 128 and n == 2 * nreg and nreg % 4 == 0
    assert not np.any(np.asarray(b)), "nonzero bias not supported"

    E = np.asarray(E, np.float32)
    edge_index = np.asarray(edge_index)
    eids = np.asarray(eids).astype(np.int32)
    # keep only E rows some eids references, then int8-quantize (dequant
    # scale folds into the L1 weights)
    used = np.zeros(etab, bool)
    used[eids.ravel()] = True
    remap = np.cumsum(used, dtype=np.int64) - 1
    eids = remap[eids].astype(np.int32)
    e_used = E[used]
    nu = e_used.shape[0]
    nsh = (nu + N_CORES - 1) // N_CORES
    etab = nsh * N_CORES
    e_scale = np.float32(max(float(np.abs(e_used).max()), 1e-30) / 127.0)
    e_q = np.zeros((etab, 128), np.int8)
    e_q[:nu] = np.clip(np.rint(e_used / e_scale), -127, 127)

    query = (np.asarray(metapath_emb, np.float32) @ np.asarray(Wq, np.float32)
             + np.asarray(bq, np.float32))
    query_scaled = query * scale

    ng1 = math.ceil(nreg / 128)
    ng2 = math.ceil((nreg // 2) / 128)

    # per-metapath: degree, dst-sorted edges
    metas = []
    for i in range(P):
        src = edge_index[i, 0].astype(np.int32)
        dst = edge_index[i, 1].astype(np.int32)
        deg = np.bincount(dst, minlength=n)
        degc = np.maximum(deg, 1).astype(np.int32)
        order = np.argsort(dst, kind="stable")
        metas.append((degc, src[order], dst[order]))

    def rng(i, lo, hi):
        _, ssrc, sdst = metas[i]
        a, bb = np.searchsorted(sdst, [lo, hi])
        return ssrc[a:bb], sdst[a:bb]

    spans = []
    for c in range(N_CORES):
        i, h = c // 2, c % 2
        lo1, lo2 = h * nreg, h * (nreg // 2)
        spans.append((rng(i, lo1, lo1 + ng1 * 128),
                      rng(i, lo2, lo2 + ng2 * 128), lo1, lo2))

    nb1 = max(1, max(math.ceil(_group_max(s[0][1], s[2], ng1) / 128)
                     for s in spans))
    nb2 = max(1, max(math.ceil(_group_max(s[1][1], s[3], ng2) / 128)
                     for s in spans))

    iota = np.tile(np.arange(128, dtype=np.float32), (128, 1))
    ident = np.eye(128, dtype=np.float32)

    in_maps = []
    for c in range(N_CORES):
        i, h = c // 2, c % 2
        (s1, d1), (s2, d2), lo1, lo2 = spans[c]
        degc = metas[i][0]
        # L1 gathers from the E table: compose indices through eids.
        lo16a, pk8a, dl8a = _build_grids(eids[i][s1], d1, lo1, ng1, nb1, degc)
        # L2 gathers from x1_full: indices are node ids.
        lo16b, pk8b, dl8b = _build_grids(s2, d2, lo2, ng2, nb2, degc)
        rows1 = np.minimum(lo1 + 128 * np.arange(ng1)[None, :]
                           + np.arange(128)[:, None], n - 1)
        idxd1 = eids[i][rows1].astype(np.int32)
        idxd2 = np.minimum(lo2 + 128 * np.arange(ng2)[None, :]
                           + np.arange(128)[:, None], n - 1).astype(np.int32)
        selm = np.zeros((128, 4), np.float32)
        selm[:, i] = 1.0
        wmat = np.concatenate([
            (np.asarray(W_rel[i, 0], np.float32) * e_scale),
            (np.asarray(W_root[i, 0], np.float32) * e_scale),
            np.asarray(W_rel[i, 1], np.float32),
            np.asarray(W_root[i, 1], np.float32),
            np.tile(query_scaled[i], (128, 1)).astype(np.float32),
            ident,
        ], axis=1).astype(bf16)
        in_maps.append(dict(
            e_sh=np.ascontiguousarray(e_q[c * nsh:(c + 1) * nsh]),
            g_lo=np.concatenate([lo16a, lo16b], axis=1),
            g_u8=np.concatenate([pk8a, dl8a, pk8b, dl8b], axis=1),
            g_xd=np.concatenate([idxd1, idxd2], axis=1).astype(np.int32),
            cst=np.concatenate([iota, selm], axis=1).astype(np.float32),
            wts=wmat,
        ))

    nc = build_program(n, nreg, etab, ng1, nb1, ng2, nb2)
    nc.compile()
    kernel.last_nc = nc
    kernel.last_in_maps = in_maps
    res = run_bass_kernel_spmd(nc, in_maps, core_ids=list(range(N_CORES)),
                               trace=trace)

    q = nreg // 2
    a_rows = np.concatenate(
        [res.results[c]["out_part"].astype(np.float32) for c in (0, 2, 4, 6)],
        axis=0)[:q]
    b_rows = np.concatenate(
        [res.results[c]["out_part"].astype(np.float32) for c in (1, 3, 5, 7)],
        axis=0)[:q]
    out = np.concatenate([a_rows, b_rows], axis=0).astype(np.float32)
    kernel.last_results = res
    return out


# revision 31
# speedup vs baseline: 1.1322x; 1.1322x over previous
"""HAN layer (4 metapaths x 2-layer mean-RGCN + metapath attention) on 8 trn2 cores.

Sharding: cores (2i, 2i+1) handle metapath i, splitting 128-dst blocks by
PARITY (even blocks -> core 2i, odd -> 2i+1) for BOTH layers. With node rows
stored in parity-permuted order (perm(j) = side*ng1*128 + (blk>>1)*128 + pos),
layer-2's edge grid is exactly the first ng2 groups of layer-1's grid — the
same bytes serve both layers, and both gather tables (x0perm, x1_full) share
the perm layout so gather indices coincide. Attention: score AllGather +
ReduceScatter over {0,2,4,6} / {1,3,5,7}; host interleaves blocks back.

Device algorithm per layer (linearity: segment_sum(x[src]) @ Wm): edges are
host-sorted into 128-dst groups; an indirect DMA gathers x[src] rows for a
group; per 128-edge chunk a selector eq[e,d] = (dl[e]==d)*rec[e] is built on
DVE and matmul-accumulated on PE into meanT = (segment_mean)^T in PSUM; two
dense matmuls + fused ReLU produce the group's 128 output rows, written
contiguously (no scatter anywhere).

Host->device transfer dominates (narrow tunnel, ~44MB/s, no compression, big
per-buffer fixed cost): E is int8-quantized (scale folded into L1 weights),
compacted to used rows, sharded 1/8 + device AllGather; x0perm = E[eids] is
materialized on device from a shipped index list; grids are 4B/slot (u16
idx-lo + u8 [idx-hi<<6|deg] + u8 dst-local) unpacked on DVE; 6 input buffers
total; all compute bf16 with f32 PSUM.
"""

import math
import numpy as np

try:
    # run_bass_kernel_spmd re-jits an identical XLA wrapper on every call;
    # the persistent compilation cache makes those re-jits near-free.
    import jax as _jax
    _jax.config.update("jax_compilation_cache_dir", "/tmp/jax_cc")
    _jax.config.update("jax_persistent_cache_min_entry_size_bytes", -1)
    _jax.config.update("jax_persistent_cache_min_compile_time_secs", 0.0)
except Exception:
    pass

import concourse.bass as bass
import concourse.bacc as bacc
import concourse.mybir as mybir
from concourse.tile import TileContext
from concourse.bass_utils import run_bass_kernel_spmd

F32 = mybir.dt.float32
BF16 = mybir.dt.bfloat16
I32 = mybir.dt.int32
I8 = mybir.dt.int8
U16 = mybir.dt.uint16
U8 = mybir.dt.uint8

N_CORES = 8
BF = 4     # output groups batched per store DMA
CH = 16    # groups per grid-load DMA


def _np_bf16():
    import ml_dtypes
    return ml_dtypes.bfloat16


# ----------------------------------------------------------------- host prep

def _build_grids(idxs, gidx, dl, degv, ng, nb):
    """Packed grid for dst-sorted edges with group ids gidx (non-decreasing)
    and dst-local ids dl. Slot j = p*nb + b of group g lands at partition p,
    column g*nb + b. Ships u16 idx-low + u8 (idx-high<<6 | deg) + u8 dl.
    Empty slots: dl=128 (selector row all-zero), pk=1 (finite reciprocal)."""
    assert degv.size == 0 or degv.max() <= 63
    assert idxs.size == 0 or idxs.max() < (1 << 18)
    starts = np.searchsorted(gidx, np.arange(ng))
    slot = np.arange(len(gidx)) - starts[gidx]
    p = slot & 127
    b = slot >> 7
    col = gidx * nb + b
    S = nb * ng
    lo16 = np.zeros((128, S), np.uint16)
    pk8 = np.ones((128, S), np.uint8)
    dl8 = np.full((128, S), 128, np.uint8)
    lo16[p, col] = (idxs & 0xFFFF).astype(np.uint16)
    pk8[p, col] = (((idxs >> 16) << 6) | degv).astype(np.uint8)
    dl8[p, col] = dl.astype(np.uint8)
    return lo16, pk8, dl8


# ------------------------------------------------------------- device build

def _emit_layer(nc, tc, pools, table, table_i8, n_hi, glo, gu8, dl_off,
                gidxd, xd_off, wm_t, wr_t, ng, nb, iota_t, ident_t,
                out_dram, rows_total, hook=None):
    sb, sbg, psum, sbeq = pools
    stage = None
    for g in range(ng):
        if g % CH == 0:
            w = min(CH, ng - g)
            lot = sbg.tile([128, nb * w], U16, tag="lot")
            nc.sync.dma_start(out=lot[:], in_=glo[:, g * nb:(g + w) * nb])
            pkt = sbg.tile([128, nb * w], U8, tag="pkt")
            nc.sync.dma_start(out=pkt[:], in_=gu8[:, g * nb:(g + w) * nb])
            dlt8 = sbg.tile([128, nb * w], U8, tag="dlt8")
            nc.sync.dma_start(
                out=dlt8[:], in_=gu8[:, dl_off + g * nb:dl_off + (g + w) * nb])
            idxdt = sbg.tile([128, w], I32, tag="idxdt")
            nc.sync.dma_start(out=idxdt[:],
                              in_=gidxd[:, xd_off + g:xd_off + g + w])
            # unpack pk = hi<<6 | deg without mod: 64*hi via is_ge steps,
            # idx = lo + 65536*hi (exact in f32: < 2^24), rec = 1/deg
            pkf = sbg.tile([128, nb * w], F32, tag="pkf")
            nc.vector.tensor_copy(out=pkf[:], in_=pkt[:])
            hi64 = sbg.tile([128, nb * w], F32, tag="hi64")
            nc.vector.tensor_scalar(out=hi64[:], in0=pkf[:], scalar1=64.0,
                                    scalar2=64.0, op0=mybir.AluOpType.is_ge,
                                    op1=mybir.AluOpType.mult)
            for k in range(1, n_hi):
                hpart = sbg.tile([128, nb * w], F32, tag="hpart")
                nc.vector.tensor_scalar(
                    out=hpart[:], in0=pkf[:], scalar1=64.0 * (k + 1),
                    scalar2=64.0, op0=mybir.AluOpType.is_ge,
                    op1=mybir.AluOpType.mult)
                nc.vector.tensor_tensor(out=hi64[:], in0=hi64[:], in1=hpart[:],
                                        op=mybir.AluOpType.add)
            dgf = sbg.tile([128, nb * w], F32, tag="dgf")
            nc.vector.tensor_tensor(out=dgf[:], in0=pkf[:], in1=hi64[:],
                                    op=mybir.AluOpType.subtract)
            lof = sbg.tile([128, nb * w], F32, tag="lof")
            nc.vector.tensor_copy(out=lof[:], in_=lot[:])
            nc.vector.tensor_scalar(out=hi64[:], in0=hi64[:], scalar1=1024.0,
                                    scalar2=None, op0=mybir.AluOpType.mult)
            idxt = sbg.tile([128, nb * w], I32, tag="idxt")
            nc.vector.tensor_tensor(out=idxt[:], in0=hi64[:], in1=lof[:],
                                    op=mybir.AluOpType.add)
            dlf = sbg.tile([128, nb * w], F32, tag="dlf")
            nc.vector.tensor_copy(out=dlf[:], in_=dlt8[:])
            recf = sbg.tile([128, nb * w], F32, tag="recf")
            nc.vector.reciprocal(out=recf[:], in_=dgf[:])
        o = (g % CH) * nb

        if table_i8:
            msgs_raw = sb.tile([128, nb * 128], I8, tag="msgs_raw")
        else:
            msgs_raw = sb.tile([128, nb * 128], BF16, tag="msgs")
        for b in range(nb):
            nc.gpsimd.indirect_dma_start(
                out=msgs_raw[:, b * 128:(b + 1) * 128], out_offset=None,
                in_=table[:],
                in_offset=bass.IndirectOffsetOnAxis(
                    ap=idxt[:, o + b:o + b + 1], axis=0))
        if table_i8:
            msgs = sb.tile([128, nb * 128], BF16, tag="msgs")
            nc.vector.tensor_copy(out=msgs[:], in_=msgs_raw[:])
        else:
            msgs = msgs_raw

        meant_ps = psum.tile([128, 128], F32, space="PSUM", tag="meant")
        for b in range(nb):
            eq = sbeq.tile([128, 128], BF16, tag="eq")
            nc.vector.tensor_scalar(
                out=eq[:], in0=iota_t[:],
                scalar1=dlf[:, o + b:o + b + 1], scalar2=recf[:, o + b:o + b + 1],
                op0=mybir.AluOpType.is_equal, op1=mybir.AluOpType.mult)
            nc.tensor.matmul(out=meant_ps[:], lhsT=msgs[:, b * 128:(b + 1) * 128],
                             rhs=eq[:], start=(b == 0), stop=(b == nb - 1))
        meant = sb.tile([128, 128], BF16, tag="meant_sb")
        nc.vector.tensor_copy(out=meant[:], in_=meant_ps[:])

        if table_i8:
            xd_raw = sb.tile([128, 128], I8, tag="xd_raw")
        else:
            xd_raw = sb.tile([128, 128], BF16, tag="xd")
        nc.gpsimd.indirect_dma_start(
            out=xd_raw[:], out_offset=None, in_=table[:],
            in_offset=bass.IndirectOffsetOnAxis(
                ap=idxdt[:, g % CH:g % CH + 1], axis=0))
        if table_i8:
            xd = sb.tile([128, 128], BF16, tag="xd")
            nc.vector.tensor_copy(out=xd[:], in_=xd_raw[:])
        else:
            xd = xd_raw
        xdt_ps = psum.tile([128, 128], BF16, space="PSUM", tag="xdt")
        nc.tensor.transpose(out=xdt_ps[:], in_=xd[:], identity=ident_t[:])
        xdt = sb.tile([128, 128], BF16, tag="xdt_sb")
        nc.vector.tensor_copy(out=xdt[:], in_=xdt_ps[:])

        h_ps = psum.tile([128, 128], F32, space="PSUM", tag="hps")
        nc.tensor.matmul(out=h_ps[:], lhsT=meant[:], rhs=wm_t[:],
                         start=True, stop=False)
        nc.tensor.matmul(out=h_ps[:], lhsT=xdt[:], rhs=wr_t[:],
                         start=False, stop=True)

        gb = g % BF
        if gb == 0:
            bw = min(BF, ng - g)
            stage = sb.tile([128, bw * 128], BF16, tag="xn_stage")
        xn = stage[:, gb * 128:(gb + 1) * 128]
        nc.scalar.activation(out=xn, in_=h_ps[:],
                             func=mybir.ActivationFunctionType.Relu)
        if hook is not None:
            hook(g, xn)
        if gb == bw - 1:
            g0 = g - gb
            rows = (gb + 1) * 128
            nc.sync.dma_start(
                out=out_dram[g0 * 128:g0 * 128 + rows, :]
                .rearrange("(a t) f -> t a f", t=128),
                in_=stage[:, :rows].rearrange("p (a f) -> p a f", f=128))


def build_program(etab, ng1, nb1, ng2):
    nc = bacc.Bacc("TRN2", target_bir_lowering=False, debug=False,
                   num_devices=N_CORES)
    nsh = etab // N_CORES
    tb = 2 * ng1                 # total 128-blocks in perm layout
    nperm = ng1 * 128            # rows per parity side
    nrs = (ng2 * 128) // 4       # ReduceScatter rows per rank
    S1 = nb1 * ng1

    def ei(name, shape, dt=F32):
        return nc.dram_tensor(name, shape, dt, kind="ExternalInput")

    # consolidated inputs (per-buffer transfer overhead is large)
    e_sh = ei("e_sh", [nsh, 128], I8)
    g_lo = ei("g_lo", [128, S1], U16)
    g_u8 = ei("g_u8", [128, 2 * S1], U8)       # [pk | dl]
    g_xd = ei("g_xd", [128, ng1 + ng2 + tb], I32)  # [idxd1 | idxd2 | eip]
    cst = ei("cst", [128, 132])                # [iota | sel]
    wts = ei("wts", [128, 6 * 128], BF16)      # [wm1|wr1|wm2|wr2|qs|ident]

    out_part = nc.dram_tensor("out_part", [nrs, 128], BF16,
                              kind="ExternalOutput")

    e_loc = nc.dram_tensor("e_loc", [nsh, 128], I8)
    e_full = nc.dram_tensor("e_full", [nsh * N_CORES, 128], I8)
    x0p = nc.dram_tensor("x0p", [tb * 128, 128], I8)
    x1_half = nc.dram_tensor("x1_half", [nperm, 128], BF16)
    x1_full = nc.dram_tensor("x1_full", [2 * nperm, 128], BF16)
    x2b = nc.dram_tensor("x2b", [ng2 * 128, 128], BF16)
    sc_in = nc.dram_tensor("sc_in", [ng2, 128], F32)
    sc_all = nc.dram_tensor("sc_all", [4 * ng2, 128], F32)
    rs_in = nc.dram_tensor("rs_in", [ng2 * 128, 128], BF16)
    rs_out = nc.dram_tensor("rs_out", [nrs, 128], BF16)

    all_group = [list(range(N_CORES))]
    pair_groups = [[2 * i, 2 * i + 1] for i in range(4)]
    attn_groups = [[0, 2, 4, 6], [1, 3, 5, 7]]

    with TileContext(nc) as tc:
        with (
            tc.tile_pool(name="const", bufs=1) as cpool,
            tc.tile_pool(name="sb", bufs=3) as sb,
            tc.tile_pool(name="sbg", bufs=2) as sbg,
            tc.tile_pool(name="sbeq", bufs=4) as sbeq,
            tc.tile_pool(name="psum", bufs=2, space="PSUM") as psum,
        ):
            def cload(src, c0, cols, tag, dt):
                t = cpool.tile([128, cols], dt, tag=tag)
                nc.sync.dma_start(out=t[:], in_=src[:, c0:c0 + cols])
                return t

            iota_t = cload(cst, 0, 128, "c_iota", F32)
            sel_t = cload(cst, 128, 4, "c_sel", F32)
            wm1_t = cload(wts, 0, 128, "c_wm1", BF16)
            wr1_t = cload(wts, 128, 128, "c_wr1", BF16)
            wm2_t = cload(wts, 256, 128, "c_wm2", BF16)
            wr2_t = cload(wts, 384, 128, "c_wr2", BF16)
            qs_t = cload(wts, 512, 128, "c_qs", BF16)
            ident_t = cload(wts, 640, 128, "c_ident", BF16)
            score_sb = cpool.tile([128, ng2], F32, tag="c_score")

            pools = (sb, sbg, psum, sbeq)

            nc.sync.dma_start(out=e_loc[:, :], in_=e_sh[:, :])
            nc.gpsimd.collective_compute(
                "AllGather", mybir.AluOpType.bypass,
                replica_groups=all_group,
                ins=[e_loc[:, :]], outs=[e_full[:, :]])

            # materialize x0perm = E[eids] in parity-permuted block order
            for c0 in range(0, tb, CH):
                w = min(CH, tb - c0)
                eipt = sbg.tile([128, w], I32, tag="eipt")
                nc.sync.dma_start(
                    out=eipt[:],
                    in_=g_xd[:, ng1 + ng2 + c0:ng1 + ng2 + c0 + w])
                for j in range(w):
                    xt = sb.tile([128, 128], I8, tag="x0p_t")
                    nc.gpsimd.indirect_dma_start(
                        out=xt[:], out_offset=None, in_=e_full[:],
                        in_offset=bass.IndirectOffsetOnAxis(
                            ap=eipt[:, j:j + 1], axis=0))
                    nc.sync.dma_start(
                        out=x0p[(c0 + j) * 128:(c0 + j + 1) * 128, :],
                        in_=xt[:])

            _emit_layer(nc, tc, pools, x0p, True, 1, g_lo, g_u8, S1,
                        g_xd, 0, wm1_t, wr1_t, ng1, nb1, iota_t, ident_t,
                        x1_half, nperm)

            nc.gpsimd.collective_compute(
                "AllGather", mybir.AluOpType.bypass,
                replica_groups=pair_groups,
                ins=[x1_half[:, :]], outs=[x1_full[:, :]])

            def score_hook(g, xn):
                t = sb.tile([128, 128], F32, tag="sc_tmp")
                nc.vector.tensor_tensor(out=t[:], in0=xn, in1=qs_t[:],
                                        op=mybir.AluOpType.mult)
                nc.vector.reduce_sum(out=score_sb[:, g:g + 1], in_=t[:],
                                     axis=mybir.AxisListType.X)

            # L2 reuses the first ng2 groups of the L1 grid verbatim
            _emit_layer(nc, tc, pools, x1_full, False, 1, g_lo, g_u8, S1,
                        g_xd, ng1, wm2_t, wr2_t, ng2, nb1, iota_t, ident_t,
                        x2b, ng2 * 128, hook=score_hook)

            nc.sync.dma_start(out=sc_in[:, :].rearrange("t p -> p t"),
                              in_=score_sb[:, :])
            nc.gpsimd.collective_compute(
                "AllGather", mybir.AluOpType.bypass,
                replica_groups=attn_groups,
                ins=[sc_in[:, :]], outs=[sc_all[:, :]])

            # softmax over 4 metapaths (elementwise across four [128,ng2] tiles)
            s_t = []
            for p in range(4):
                st = cpool.tile([128, ng2], F32, tag=f"s{p}")
                nc.sync.dma_start(
                    out=st[:],
                    in_=sc_all[p * ng2:(p + 1) * ng2, :].rearrange("t p -> p t"))
                s_t.append(st)
            m = cpool.tile([128, ng2], F32, tag="c_m")
            nc.vector.tensor_tensor(out=m[:], in0=s_t[0][:], in1=s_t[1][:],
                                    op=mybir.AluOpType.max)
            for p in (2, 3):
                nc.vector.tensor_tensor(out=m[:], in0=m[:], in1=s_t[p][:],
                                        op=mybir.AluOpType.max)
            e_t = []
            for p in range(4):
                dt_ = cpool.tile([128, ng2], F32, tag=f"d{p}")
                nc.vector.tensor_tensor(out=dt_[:], in0=s_t[p][:], in1=m[:],
                                        op=mybir.AluOpType.subtract)
                et = cpool.tile([128, ng2], F32, tag=f"e{p}")
                nc.scalar.activation(out=et[:], in_=dt_[:],
                                     func=mybir.ActivationFunctionType.Exp)
                e_t.append(et)
            z = cpool.tile([128, ng2], F32, tag="c_z")
            nc.vector.tensor_tensor(out=z[:], in0=e_t[0][:], in1=e_t[1][:],
                                    op=mybir.AluOpType.add)
            for p in (2, 3):
                nc.vector.tensor_tensor(out=z[:], in0=z[:], in1=e_t[p][:],
                                        op=mybir.AluOpType.add)
            rz = cpool.tile([128, ng2], F32, tag="c_rz")
            nc.vector.reciprocal(out=rz[:], in_=z[:])
            wown = cpool.tile([128, ng2], F32, tag="c_wown")
            acc = cpool.tile([128, ng2], F32, tag="c_acc")
            nc.vector.tensor_scalar(out=wown[:], in0=e_t[0][:],
                                    scalar1=sel_t[:, 0:1], scalar2=None,
                                    op0=mybir.AluOpType.mult)
            for p in (1, 2, 3):
                nc.vector.tensor_scalar(out=acc[:], in0=e_t[p][:],
                                        scalar1=sel_t[:, p:p + 1], scalar2=None,
                                        op0=mybir.AluOpType.mult)
                nc.vector.tensor_tensor(out=wown[:], in0=wown[:], in1=acc[:],
                                        op=mybir.AluOpType.add)
            nc.vector.tensor_tensor(out=wown[:], in0=wown[:], in1=rz[:],
                                    op=mybir.AluOpType.mult)

            # weighted partials, batched BF groups per DMA
            for g0 in range(0, ng2, BF):
                bw = min(BF, ng2 - g0)
                xt = sb.tile([128, bw * 128], BF16, tag="attn_x")
                nc.sync.dma_start(
                    out=xt[:].rearrange("p (a f) -> p a f", f=128),
                    in_=x2b[g0 * 128:(g0 + bw) * 128, :]
                    .rearrange("(a t) f -> t a f", t=128))
                wt = sb.tile([128, bw * 128], BF16, tag="attn_w")
                for j in range(bw):
                    nc.vector.tensor_scalar(
                        out=wt[:, j * 128:(j + 1) * 128],
                        in0=xt[:, j * 128:(j + 1) * 128],
                        scalar1=wown[:, g0 + j:g0 + j + 1], scalar2=None,
                        op0=mybir.AluOpType.mult)
                nc.sync.dma_start(
                    out=rs_in[g0 * 128:(g0 + bw) * 128, :]
                    .rearrange("(a t) f -> t a f", t=128),
                    in_=wt[:].rearrange("p (a f) -> p a f", f=128))

            nc.gpsimd.collective_compute(
                "ReduceScatter", mybir.AluOpType.add,
                replica_groups=attn_groups,
                ins=[rs_in[:, :]], outs=[rs_out[:, :]])

            # rs_out [nrs,128] -> out_part, bounced through SBUF
            nblk = nrs // 128
            fin = cpool.tile([128, nblk * 128], BF16, tag="c_fin")
            nc.sync.dma_start(
                out=fin[:].rearrange("p (a f) -> p a f", f=128),
                in_=rs_out[:, :].rearrange("(a t) f -> t a f", t=128))
            nc.sync.dma_start(
                out=out_part[:, :].rearrange("(a t) f -> t a f", t=128),
                in_=fin[:].rearrange("p (a f) -> p a f", f=128))
    return nc


# ----------------------------------------------------------------- kernel()

def kernel(E, metapath_emb, W_root, W_rel, b, Wq, bq, edge_index, eids,
           nreg=50000, trace=False, debug=False):
    bf16 = _np_bf16()
    P = edge_index.shape[0]
    n = eids.shape[1]
    d = E.shape[1]
    etab = E.shape[0]
    scale = np.float32(1.0 / math.sqrt(d))
    assert P == 4 and d == 128 and n == 2 * nreg

    E = np.asarray(E, np.float32)
    edge_index = np.asarray(edge_index)
    eids = np.asarray(eids).astype(np.int32)
    assert not np.any(np.asarray(b)), "nonzero bias not supported"

    # keep only E rows some eids references, then int8-quantize (dequant
    # scale folds into the L1 weights)
    used = np.zeros(etab, bool)
    used[eids.ravel()] = True
    remap = np.cumsum(used, dtype=np.int64) - 1
    eids = remap[eids].astype(np.int32)
    e_used = E[used]
    nu = e_used.shape[0]
    nsh = (nu + N_CORES - 1) // N_CORES
    etab = nsh * N_CORES
    e_scale = np.float32(max(float(np.abs(e_used).max()), 1e-30) / 127.0)
    e_q = np.zeros((etab, 128), np.int8)
    e_q[:nu] = np.clip(np.rint(e_used / e_scale), -127, 127)

    query = (np.asarray(metapath_emb, np.float32) @ np.asarray(Wq, np.float32)
             + np.asarray(bq, np.float32))
    query_scaled = query * scale

    tbh = math.ceil(n / 256)     # 128-blocks per parity side
    ng1 = tbh                    # L1 groups per core (one parity side)
    ng2 = math.ceil(nreg / 256)  # L2 groups per core
    assert ng2 <= ng1
    nperm = ng1 * 128
    tb = 2 * ng1

    # perm(j): parity-split node order
    def perm(j):
        blk = j >> 7
        pos = j & 127
        return (blk & 1) * nperm + (blk >> 1) * 128 + pos

    # per-metapath: degree, edges split by dst-block parity, sorted by dst
    metas = []
    for i in range(P):
        src = edge_index[i, 0].astype(np.int64)
        dst = edge_index[i, 1].astype(np.int64)
        deg = np.bincount(dst, minlength=n)
        degc = np.maximum(deg, 1).astype(np.int64)
        order = np.argsort(dst, kind="stable")
        ssrc, sdst = src[order], dst[order]
        side = (sdst >> 7) & 1
        per_h = []
        for h in (0, 1):
            m = side == h
            es, ed = ssrc[m], sdst[m]
            per_h.append((es, ed))
        metas.append((degc, per_h))

    # nb1 = max blocks over every (core, group)
    nb1 = 1
    counts = []
    for i in range(P):
        for h in (0, 1):
            es, ed = metas[i][1][h]
            gidx = (ed >> 8)                      # (blk>>1): group id
            cnt = np.bincount(gidx, minlength=ng1)
            counts.append(cnt)
            nb1 = max(nb1, math.ceil(cnt.max() / 128))
    S1 = nb1 * ng1

    iota = np.tile(np.arange(128, dtype=np.float32), (128, 1))
    ident = np.eye(128, dtype=np.float32)
    pos_col = np.arange(128, dtype=np.int64)[:, None]

    in_maps = []
    for c in range(N_CORES):
        i, h = c // 2, c % 2
        degc, per_h = metas[i]
        es, ed = per_h[h]
        gidx = (ed >> 8).astype(np.int64)
        dl = ed & 127
        lo16, pk8, dl8 = _build_grids(perm(es), gidx, dl, degc[ed], ng1, nb1)
        # xd index tables (into the perm-ordered tables)
        gl1 = np.arange(ng1)[None, :]
        idxd1 = (h * nperm + gl1 * 128 + pos_col).astype(np.int32)
        gl2 = np.arange(ng2)[None, :]
        idxd2 = (h * nperm + gl2 * 128 + pos_col).astype(np.int32)
        # x0perm materialization indices: block c -> global block
        cb = np.arange(tb)[None, :]
        gblk = np.where(cb < ng1, 2 * cb, 2 * (cb - ng1) + 1)
        jj = np.minimum(gblk * 128 + pos_col, n - 1)
        eip = eids[i][jj].astype(np.int32)
        selm = np.zeros((128, 4), np.float32)
        selm[:, i] = 1.0
        wmat = np.concatenate([
            (np.asarray(W_rel[i, 0], np.float32) * e_scale),
            (np.asarray(W_root[i, 0], np.float32) * e_scale),
            np.asarray(W_rel[i, 1], np.float32),
            np.asarray(W_root[i, 1], np.float32),
            np.tile(query_scaled[i], (128, 1)).astype(np.float32),
            ident,
        ], axis=1).astype(bf16)
        in_maps.append(dict(
            e_sh=np.ascontiguousarray(e_q[c * nsh:(c + 1) * nsh]),
            g_lo=lo16,
            g_u8=np.concatenate([pk8, dl8], axis=1),
            g_xd=np.concatenate([idxd1, idxd2, eip], axis=1).astype(np.int32),
            cst=np.concatenate([iota, selm], axis=1).astype(np.float32),
            wts=wmat,
        ))

    nc = build_program(etab, ng1, nb1, ng2)
    nc.compile()
    kernel.last_nc = nc
    kernel.last_in_maps = in_maps
    res = run_bass_kernel_spmd(nc, in_maps, core_ids=list(range(N_CORES)),
                               trace=trace)

    # interleave parity sides back to node order
    a_rows = np.concatenate(
        [res.results[c]["out_part"].astype(np.float32) for c in (0, 2, 4, 6)],
        axis=0).reshape(ng2, 128, 128)
    b_rows = np.concatenate(
        [res.results[c]["out_part"].astype(np.float32) for c in (1, 3, 5, 7)],
        axis=0).reshape(ng2, 128, 128)
    full = np.empty((2 * ng2, 128, 128), np.float32)
    full[0::2] = a_rows
    full[1::2] = b_rows
    out = full.reshape(-1, 128)[:nreg].astype(np.float32)
    kernel.last_results = res
    return out


# revision 39
# speedup vs baseline: 1.1663x; 1.0301x over previous
"""HAN layer (4 metapaths x 2-layer mean-RGCN + metapath attention) on 8 trn2 cores.

Sharding: cores (2i, 2i+1) handle metapath i, splitting 128-dst blocks by
PARITY (even blocks -> core 2i, odd -> 2i+1) for BOTH layers. With node rows
stored in parity-permuted order (perm(j) = side*ng1*128 + (blk>>1)*128 + pos),
layer-2's edge grid is exactly the first ng2 groups of layer-1's grid — the
same bytes serve both layers, and both gather tables (x0perm, x1_full) share
the perm layout so gather indices coincide. Attention: score AllGather +
ReduceScatter over {0,2,4,6} / {1,3,5,7}; host interleaves blocks back.

Device algorithm per layer (linearity: segment_sum(x[src]) @ Wm): edges are
host-sorted into 128-dst groups; an indirect DMA gathers x[src] rows for a
group; per 128-edge chunk a selector eq[e,d] = (dl[e]==d)*rec[e] is built on
DVE and matmul-accumulated on PE into meanT = (segment_mean)^T in PSUM; two
dense matmuls + fused ReLU produce the group's 128 output rows, written
contiguously (no scatter anywhere).

Host->device transfer dominates (narrow tunnel, ~44MB/s, no compression, big
per-buffer fixed cost): E is int8-quantized (scale folded into L1 weights),
compacted to used rows, sharded 1/8 + device AllGather; x0perm = E[eids] is
materialized on device from a shipped index list; grids are 4B/slot (u16
idx-lo + u8 [idx-hi<<6|deg] + u8 dst-local) unpacked on DVE; 6 input buffers
total; all compute bf16 with f32 PSUM.
"""

import math
import numpy as np

try:
    # run_bass_kernel_spmd re-jits an identical XLA wrapper on every call;
    # the persistent compilation cache makes those re-jits near-free.
    import jax as _jax
    _jax.config.update("jax_compilation_cache_dir", "/tmp/jax_cc")
    _jax.config.update("jax_persistent_cache_min_entry_size_bytes", -1)
    _jax.config.update("jax_persistent_cache_min_compile_time_secs", 0.0)
except Exception:
    pass

import concourse.bass as bass
import concourse.bacc as bacc
import concourse.mybir as mybir
from concourse.tile import TileContext
from concourse.bass_utils import run_bass_kernel_spmd

F32 = mybir.dt.float32
BF16 = mybir.dt.bfloat16
I32 = mybir.dt.int32
I8 = mybir.dt.int8
U16 = mybir.dt.uint16
U8 = mybir.dt.uint8

N_CORES = 8
BF = 4     # output groups batched per store DMA
CH = 16    # groups per grid-load DMA


def _np_bf16():
    import ml_dtypes
    return ml_dtypes.bfloat16


# ----------------------------------------------------------------- host prep

def _build_grids(idxs, gidx, dl, degv, ng, nb):
    """Packed grid for dst-sorted edges with group ids gidx (non-decreasing)
    and dst-local ids dl. Slot j = p*nb + b of group g lands at partition p,
    column g*nb + b. Ships u16 idx-low + u8 (idx-high<<6 | deg) + u8 dl.
    Empty slots: dl=128 (selector row all-zero), pk=1 (finite reciprocal)."""
    assert degv.size == 0 or degv.max() <= 63
    assert idxs.size == 0 or idxs.max() < (1 << 18)
    starts = np.searchsorted(gidx, np.arange(ng))
    slot = np.arange(len(gidx)) - starts[gidx]
    p = slot & 127
    b = slot >> 7
    col = gidx * nb + b
    S = nb * ng
    lo16 = np.zeros((128, S), np.uint16)
    pk8 = np.ones((128, S), np.uint8)
    dl8 = np.full((128, S), 128, np.uint8)
    lo16[p, col] = (idxs & 0xFFFF).astype(np.uint16)
    pk8[p, col] = (((idxs >> 16) << 6) | degv).astype(np.uint8)
    dl8[p, col] = dl.astype(np.uint8)
    return lo16, pk8, dl8


# ------------------------------------------------------------- device build

def _emit_layer(nc, tc, pools, table, table_i8, n_hi, glo, gu8, dl_off,
                basep_t, wm_t, wr_t, ng, nb, iota_t, ident_t,
                out_dram, rows_total, hook=None):
    sb, sbg, psum, sbeq = pools
    stage = None
    for g in range(ng):
        if g % CH == 0:
            w = min(CH, ng - g)
            lot = sbg.tile([128, nb * w], U16, tag="lot")
            nc.sync.dma_start(out=lot[:], in_=glo[:, g * nb:(g + w) * nb])
            pkt = sbg.tile([128, nb * w], U8, tag="pkt")
            nc.sync.dma_start(out=pkt[:], in_=gu8[:, g * nb:(g + w) * nb])
            dlt8 = sbg.tile([128, nb * w], U8, tag="dlt8")
            nc.sync.dma_start(
                out=dlt8[:], in_=gu8[:, dl_off + g * nb:dl_off + (g + w) * nb])
            # xd rows are h*nperm + 128*(g+j) + p in both layers' tables:
            # build on DVE from the shipped per-partition base column
            idxdf = sbg.tile([128, w], F32, tag="idxdf")
            nc.vector.tensor_scalar(out=idxdf[:], in0=iota_t[:, :w],
                                    scalar1=128.0, scalar2=basep_t[:, 0:1],
                                    op0=mybir.AluOpType.mult,
                                    op1=mybir.AluOpType.add)
            idxdt = sbg.tile([128, w], I32, tag="idxdt")
            nc.vector.tensor_scalar(out=idxdt[:], in0=idxdf[:],
                                    scalar1=float(128 * g), scalar2=None,
                                    op0=mybir.AluOpType.add)
            # unpack pk = hi<<6 | deg without mod: 64*hi via is_ge steps,
            # idx = lo + 65536*hi (exact in f32: < 2^24), rec = 1/deg
            pkf = sbg.tile([128, nb * w], F32, tag="pkf")
            nc.vector.tensor_copy(out=pkf[:], in_=pkt[:])
            hi64 = sbg.tile([128, nb * w], F32, tag="hi64")
            nc.vector.tensor_scalar(out=hi64[:], in0=pkf[:], scalar1=64.0,
                                    scalar2=64.0, op0=mybir.AluOpType.is_ge,
                                    op1=mybir.AluOpType.mult)
            for k in range(1, n_hi):
                hpart = sbg.tile([128, nb * w], F32, tag="hpart")
                nc.vector.tensor_scalar(
                    out=hpart[:], in0=pkf[:], scalar1=64.0 * (k + 1),
                    scalar2=64.0, op0=mybir.AluOpType.is_ge,
                    op1=mybir.AluOpType.mult)
                nc.vector.tensor_tensor(out=hi64[:], in0=hi64[:], in1=hpart[:],
                                        op=mybir.AluOpType.add)
            dgf = sbg.tile([128, nb * w], F32, tag="dgf")
            nc.vector.tensor_tensor(out=dgf[:], in0=pkf[:], in1=hi64[:],
                                    op=mybir.AluOpType.subtract)
            lof = sbg.tile([128, nb * w], F32, tag="lof")
            nc.vector.tensor_copy(out=lof[:], in_=lot[:])
            nc.vector.tensor_scalar(out=hi64[:], in0=hi64[:], scalar1=1024.0,
                                    scalar2=None, op0=mybir.AluOpType.mult)
            idxt = sbg.tile([128, nb * w], I32, tag="idxt")
            nc.vector.tensor_tensor(out=idxt[:], in0=hi64[:], in1=lof[:],
                                    op=mybir.AluOpType.add)
            dlf = sbg.tile([128, nb * w], F32, tag="dlf")
            nc.vector.tensor_copy(out=dlf[:], in_=dlt8[:])
            recf = sbg.tile([128, nb * w], F32, tag="recf")
            nc.vector.reciprocal(out=recf[:], in_=dgf[:])
        o = (g % CH) * nb

        if table_i8:
            msgs_raw = sb.tile([128, nb * 128], I8, tag="msgs_raw")
        else:
            msgs_raw = sb.tile([128, nb * 128], BF16, tag="msgs")
        for b in range(nb):
            nc.gpsimd.indirect_dma_start(
                out=msgs_raw[:, b * 128:(b + 1) * 128], out_offset=None,
                in_=table[:],
                in_offset=bass.IndirectOffsetOnAxis(
                    ap=idxt[:, o + b:o + b + 1], axis=0))
        if table_i8:
            msgs = sb.tile([128, nb * 128], BF16, tag="msgs")
            nc.vector.tensor_copy(out=msgs[:], in_=msgs_raw[:])
        else:
            msgs = msgs_raw

        meant_ps = psum.tile([128, 128], F32, space="PSUM", tag="meant")
        for b in range(nb):
            eq = sbeq.tile([128, 128], BF16, tag="eq")
            nc.vector.tensor_scalar(
                out=eq[:], in0=iota_t[:],
                scalar1=dlf[:, o + b:o + b + 1], scalar2=recf[:, o + b:o + b + 1],
                op0=mybir.AluOpType.is_equal, op1=mybir.AluOpType.mult)
            nc.tensor.matmul(out=meant_ps[:], lhsT=msgs[:, b * 128:(b + 1) * 128],
                             rhs=eq[:], start=(b == 0), stop=(b == nb - 1))
        meant = sb.tile([128, 128], BF16, tag="meant_sb")
        nc.vector.tensor_copy(out=meant[:], in_=meant_ps[:])

        if table_i8:
            xd_raw = sb.tile([128, 128], I8, tag="xd_raw")
        else:
            xd_raw = sb.tile([128, 128], BF16, tag="xd")
        nc.gpsimd.indirect_dma_start(
            out=xd_raw[:], out_offset=None, in_=table[:],
            in_offset=bass.IndirectOffsetOnAxis(
                ap=idxdt[:, g % CH:g % CH + 1], axis=0))
        if table_i8:
            xd = sb.tile([128, 128], BF16, tag="xd")
            nc.vector.tensor_copy(out=xd[:], in_=xd_raw[:])
        else:
            xd = xd_raw
        xdt_ps = psum.tile([128, 128], BF16, space="PSUM", tag="xdt")
        nc.tensor.transpose(out=xdt_ps[:], in_=xd[:], identity=ident_t[:])
        xdt = sb.tile([128, 128], BF16, tag="xdt_sb")
        nc.vector.tensor_copy(out=xdt[:], in_=xdt_ps[:])

        h_ps = psum.tile([128, 128], F32, space="PSUM", tag="hps")
        nc.tensor.matmul(out=h_ps[:], lhsT=meant[:], rhs=wm_t[:],
                         start=True, stop=False)
        nc.tensor.matmul(out=h_ps[:], lhsT=xdt[:], rhs=wr_t[:],
                         start=False, stop=True)

        gb = g % BF
        if gb == 0:
            bw = min(BF, ng - g)
            stage = sb.tile([128, bw * 128], BF16, tag="xn_stage")
        xn = stage[:, gb * 128:(gb + 1) * 128]
        nc.scalar.activation(out=xn, in_=h_ps[:],
                             func=mybir.ActivationFunctionType.Relu)
        if hook is not None:
            hook(g, xn)
        if gb == bw - 1:
            g0 = g - gb
            rows = (gb + 1) * 128
            nc.sync.dma_start(
                out=out_dram[g0 * 128:g0 * 128 + rows, :]
                .rearrange("(a t) f -> t a f", t=128),
                in_=stage[:, :rows].rearrange("p (a f) -> p a f", f=128))


def build_program(etab, ng1, nb1, ng2):
    nc = bacc.Bacc("TRN2", target_bir_lowering=False, debug=False,
                   num_devices=N_CORES)
    nsh = etab // N_CORES
    tb = 2 * ng1                 # total 128-blocks in perm layout
    nperm = ng1 * 128            # rows per parity side
    nrs = (ng2 * 128) // 4       # ReduceScatter rows per rank
    S1 = nb1 * ng1

    def ei(name, shape, dt=F32):
        return nc.dram_tensor(name, shape, dt, kind="ExternalInput")

    # consolidated inputs (per-buffer transfer overhead is large)
    e_sh = ei("e_sh", [nsh, 128], I8)
    g_lo = ei("g_lo", [128, S1 + tb], U16)     # [grid-lo | eip-lo]
    g_u8 = ei("g_u8", [128, 2 * S1 + tb], U8)  # [pk | dl | eip-hi]
    cst = ei("cst", [128, 133])                # [iota | sel | basep]
    wts = ei("wts", [128, 6 * 128], BF16)      # [wm1|wr1|wm2|wr2|qs|ident]

    out_part = nc.dram_tensor("out_part", [nrs, 128], BF16,
                              kind="ExternalOutput")

    e_loc = nc.dram_tensor("e_loc", [nsh, 128], I8)
    e_full = nc.dram_tensor("e_full", [nsh * N_CORES, 128], I8)
    x0p = nc.dram_tensor("x0p", [tb * 128, 128], I8)
    x1_half = nc.dram_tensor("x1_half", [nperm, 128], BF16)
    x1_full = nc.dram_tensor("x1_full", [2 * nperm, 128], BF16)
    x2b = nc.dram_tensor("x2b", [ng2 * 128, 128], BF16)
    sc_in = nc.dram_tensor("sc_in", [ng2, 128], F32)
    sc_all = nc.dram_tensor("sc_all", [4 * ng2, 128], F32)
    rs_in = nc.dram_tensor("rs_in", [ng2 * 128, 128], BF16)
    rs_out = nc.dram_tensor("rs_out", [nrs, 128], BF16)

    all_group = [list(range(N_CORES))]
    pair_groups = [[2 * i, 2 * i + 1] for i in range(4)]
    attn_groups = [[0, 2, 4, 6], [1, 3, 5, 7]]

    with TileContext(nc) as tc:
        with (
            tc.tile_pool(name="const", bufs=1) as cpool,
            tc.tile_pool(name="sb", bufs=3) as sb,
            tc.tile_pool(name="sbg", bufs=2) as sbg,
            tc.tile_pool(name="sbeq", bufs=4) as sbeq,
            tc.tile_pool(name="psum", bufs=2, space="PSUM") as psum,
        ):
            def cload(src, c0, cols, tag, dt):
                t = cpool.tile([128, cols], dt, tag=tag)
                nc.sync.dma_start(out=t[:], in_=src[:, c0:c0 + cols])
                return t

            iota_t = cload(cst, 0, 128, "c_iota", F32)
            sel_t = cload(cst, 128, 4, "c_sel", F32)
            basep_t = cload(cst, 132, 1, "c_basep", F32)
            wm1_t = cload(wts, 0, 128, "c_wm1", BF16)
            wr1_t = cload(wts, 128, 128, "c_wr1", BF16)
            wm2_t = cload(wts, 256, 128, "c_wm2", BF16)
            wr2_t = cload(wts, 384, 128, "c_wr2", BF16)
            qs_t = cload(wts, 512, 128, "c_qs", BF16)
            ident_t = cload(wts, 640, 128, "c_ident", BF16)
            score_sb = cpool.tile([128, ng2], F32, tag="c_score")

            pools = (sb, sbg, psum, sbeq)

            nc.sync.dma_start(out=e_loc[:, :], in_=e_sh[:, :])
            nc.gpsimd.collective_compute(
                "AllGather", mybir.AluOpType.bypass,
                replica_groups=all_group,
                ins=[e_loc[:, :]], outs=[e_full[:, :]])

            # materialize x0perm = E[eids] in parity-permuted block order;
            # eip ships packed as u16 lo + u8 hi
            for c0 in range(0, tb, CH):
                w = min(CH, tb - c0)
                elot = sbg.tile([128, w], U16, tag="elot")
                nc.sync.dma_start(out=elot[:],
                                  in_=g_lo[:, S1 + c0:S1 + c0 + w])
                ehit = sbg.tile([128, w], U8, tag="ehit")
                nc.sync.dma_start(out=ehit[:],
                                  in_=g_u8[:, 2 * S1 + c0:2 * S1 + c0 + w])
                elof = sbg.tile([128, w], F32, tag="elof")
                nc.vector.tensor_copy(out=elof[:], in_=elot[:])
                ehif = sbg.tile([128, w], F32, tag="ehif")
                nc.vector.tensor_scalar(out=ehif[:], in0=ehit[:],
                                        scalar1=65536.0, scalar2=None,
                                        op0=mybir.AluOpType.mult)
                eipt = sbg.tile([128, w], I32, tag="eipt")
                nc.vector.tensor_tensor(out=eipt[:], in0=elof[:], in1=ehif[:],
                                        op=mybir.AluOpType.add)
                for j in range(w):
                    xt = sb.tile([128, 128], I8, tag="x0p_t")
                    nc.gpsimd.indirect_dma_start(
                        out=xt[:], out_offset=None, in_=e_full[:],
                        in_offset=bass.IndirectOffsetOnAxis(
                            ap=eipt[:, j:j + 1], axis=0))
                    nc.sync.dma_start(
                        out=x0p[(c0 + j) * 128:(c0 + j + 1) * 128, :],
                        in_=xt[:])

            _emit_layer(nc, tc, pools, x0p, True, 1, g_lo, g_u8, S1,
                        basep_t, wm1_t, wr1_t, ng1, nb1, iota_t, ident_t,
                        x1_half, nperm)

            nc.gpsimd.collective_compute(
                "AllGather", mybir.AluOpType.bypass,
                replica_groups=pair_groups,
                ins=[x1_half[:, :]], outs=[x1_full[:, :]])

            def score_hook(g, xn):
                t = sb.tile([128, 128], F32, tag="sc_tmp")
                nc.vector.tensor_tensor(out=t[:], in0=xn, in1=qs_t[:],
                                        op=mybir.AluOpType.mult)
                nc.vector.reduce_sum(out=score_sb[:, g:g + 1], in_=t[:],
                                     axis=mybir.AxisListType.X)

            # L2 reuses the first ng2 groups of the L1 grid verbatim
            _emit_layer(nc, tc, pools, x1_full, False, 1, g_lo, g_u8, S1,
                        basep_t, wm2_t, wr2_t, ng2, nb1, iota_t, ident_t,
                        x2b, ng2 * 128, hook=score_hook)

            nc.sync.dma_start(out=sc_in[:, :].rearrange("t p -> p t"),
                              in_=score_sb[:, :])
            nc.gpsimd.collective_compute(
                "AllGather", mybir.AluOpType.bypass,
                replica_groups=attn_groups,
                ins=[sc_in[:, :]], outs=[sc_all[:, :]])

            # softmax over 4 metapaths (elementwise across four [128,ng2] tiles)
            s_t = []
            for p in range(4):
                st = cpool.tile([128, ng2], F32, tag=f"s{p}")
                nc.sync.dma_start(
                    out=st[:],
                    in_=sc_all[p * ng2:(p + 1) * ng2, :].rearrange("t p -> p t"))
                s_t.append(st)
            m = cpool.tile([128, ng2], F32, tag="c_m")
            nc.vector.tensor_tensor(out=m[:], in0=s_t[0][:], in1=s_t[1][:],
                                    op=mybir.AluOpType.max)
            for p in (2, 3):
                nc.vector.tensor_tensor(out=m[:], in0=m[:], in1=s_t[p][:],
                                        op=mybir.AluOpType.max)
            e_t = []
            for p in range(4):
                dt_ = cpool.tile([128, ng2], F32, tag=f"d{p}")
                nc.vector.tensor_tensor(out=dt_[:], in0=s_t[p][:], in1=m[:],
                                        op=mybir.AluOpType.subtract)
                et = cpool.tile([128, ng2], F32, tag=f"e{p}")
                nc.scalar.activation(out=et[:], in_=dt_[:],
                                     func=mybir.ActivationFunctionType.Exp)
                e_t.append(et)
            z = cpool.tile([128, ng2], F32, tag="c_z")
            nc.vector.tensor_tensor(out=z[:], in0=e_t[0][:], in1=e_t[1][:],
                                    op=mybir.AluOpType.add)
            for p in (2, 3):
                nc.vector.tensor_tensor(out=z[:], in0=z[:], in1=e_t[p][:],
                                        op=mybir.AluOpType.add)
            rz = cpool.tile([128, ng2], F32, tag="c_rz")
            nc.vector.reciprocal(out=rz[:], in_=z[:])
            wown = cpool.tile([128, ng2], F32, tag="c_wown")
            acc = cpool.tile([128, ng2], F32, tag="c_acc")
            nc.vector.tensor_scalar(out=wown[:], in0=e_t[0][:],
                                    scalar1=sel_t[:, 0:1], scalar2=None,
                                    op0=mybir.AluOpType.mult)
            for p in (1, 2, 3):
                nc.vector.tensor_scalar(out=acc[:], in0=e_t[p][:],
                                        scalar1=sel_t[:, p:p + 1], scalar2=None,
                                        op0=mybir.AluOpType.mult)
                nc.vector.tensor_tensor(out=wown[:], in0=wown[:], in1=acc[:],
                                        op=mybir.AluOpType.add)
            nc.vector.tensor_tensor(out=wown[:], in0=wown[:], in1=rz[:],
                                    op=mybir.AluOpType.mult)

            # weighted partials, batched BF groups per DMA
            for g0 in range(0, ng2, BF):
                bw = min(BF, ng2 - g0)
                xt = sb.tile([128, bw * 128], BF16, tag="attn_x")
                nc.sync.dma_start(
                    out=xt[:].rearrange("p (a f) -> p a f", f=128),
                    in_=x2b[g0 * 128:(g0 + bw) * 128, :]
                    .rearrange("(a t) f -> t a f", t=128))
                wt = sb.tile([128, bw * 128], BF16, tag="attn_w")
                for j in range(bw):
                    nc.vector.tensor_scalar(
                        out=wt[:, j * 128:(j + 1) * 128],
                        in0=xt[:, j * 128:(j + 1) * 128],
                        scalar1=wown[:, g0 + j:g0 + j + 1], scalar2=None,
                        op0=mybir.AluOpType.mult)
                nc.sync.dma_start(
                    out=rs_in[g0 * 128:(g0 + bw) * 128, :]
                    .rearrange("(a t) f -> t a f", t=128),
                    in_=wt[:].rearrange("p (a f) -> p a f", f=128))

            nc.gpsimd.collective_compute(
                "ReduceScatter", mybir.AluOpType.add,
                replica_groups=attn_groups,
                ins=[rs_in[:, :]], outs=[rs_out[:, :]])

            # rs_out [nrs,128] -> out_part, bounced through SBUF
            nblk = nrs // 128
            fin = cpool.tile([128, nblk * 128], BF16, tag="c_fin")
            nc.sync.dma_start(
                out=fin[:].rearrange("p (a f) -> p a f", f=128),
                in_=rs_out[:, :].rearrange("(a t) f -> t a f", t=128))
            nc.sync.dma_start(
                out=out_part[:, :].rearrange("(a t) f -> t a f", t=128),
                in_=fin[:].rearrange("p (a f) -> p a f", f=128))
    return nc


# ----------------------------------------------------------------- kernel()

def kernel(E, metapath_emb, W_root, W_rel, b, Wq, bq, edge_index, eids,
           nreg=50000, trace=False, debug=False):
    bf16 = _np_bf16()
    P = edge_index.shape[0]
    n = eids.shape[1]
    d = E.shape[1]
    etab = E.shape[0]
    scale = np.float32(1.0 / math.sqrt(d))
    assert P == 4 and d == 128 and n == 2 * nreg

    E = np.asarray(E, np.float32)
    edge_index = np.asarray(edge_index)
    eids = np.asarray(eids).astype(np.int32)
    assert not np.any(np.asarray(b)), "nonzero bias not supported"

    # keep only E rows some eids references, then int8-quantize (dequant
    # scale folds into the L1 weights)
    used = np.zeros(etab, bool)
    used[eids.ravel()] = True
    remap = np.cumsum(used, dtype=np.int64) - 1
    eids = remap[eids].astype(np.int32)
    e_used = E[used]
    nu = e_used.shape[0]
    nsh = (nu + N_CORES - 1) // N_CORES
    etab = nsh * N_CORES
    e_scale = np.float32(max(float(np.abs(e_used).max()), 1e-30) / 127.0)
    e_q = np.zeros((etab, 128), np.int8)
    e_q[:nu] = np.clip(np.rint(e_used / e_scale), -127, 127)

    query = (np.asarray(metapath_emb, np.float32) @ np.asarray(Wq, np.float32)
             + np.asarray(bq, np.float32))
    query_scaled = query * scale

    tbh = math.ceil(n / 256)     # 128-blocks per parity side
    ng1 = tbh                    # L1 groups per core (one parity side)
    ng2 = math.ceil(nreg / 256)  # L2 groups per core
    assert ng2 <= ng1
    nperm = ng1 * 128
    tb = 2 * ng1

    # perm(j): parity-split node order
    def perm(j):
        blk = j >> 7
        pos = j & 127
        return (blk & 1) * nperm + (blk >> 1) * 128 + pos

    # per-metapath: degree, edges split by dst-block parity, sorted by dst
    metas = []
    for i in range(P):
        src = edge_index[i, 0].astype(np.int64)
        dst = edge_index[i, 1].astype(np.int64)
        deg = np.bincount(dst, minlength=n)
        degc = np.maximum(deg, 1).astype(np.int64)
        order = np.argsort(dst, kind="stable")
        ssrc, sdst = src[order], dst[order]
        side = (sdst >> 7) & 1
        per_h = []
        for h in (0, 1):
            m = side == h
            es, ed = ssrc[m], sdst[m]
            per_h.append((es, ed))
        metas.append((degc, per_h))

    # nb1 = max blocks over every (core, group)
    nb1 = 1
    counts = []
    for i in range(P):
        for h in (0, 1):
            es, ed = metas[i][1][h]
            gidx = (ed >> 8)                      # (blk>>1): group id
            cnt = np.bincount(gidx, minlength=ng1)
            counts.append(cnt)
            nb1 = max(nb1, math.ceil(cnt.max() / 128))
    S1 = nb1 * ng1

    iota = np.tile(np.arange(128, dtype=np.float32), (128, 1))
    ident = np.eye(128, dtype=np.float32)
    pos_col = np.arange(128, dtype=np.int64)[:, None]

    in_maps = []
    for c in range(N_CORES):
        i, h = c // 2, c % 2
        degc, per_h = metas[i]
        es, ed = per_h[h]
        gidx = (ed >> 8).astype(np.int64)
        dl = ed & 127
        lo16, pk8, dl8 = _build_grids(perm(es), gidx, dl, degc[ed], ng1, nb1)
        # x0perm materialization indices: block c -> global block
        cb = np.arange(tb)[None, :]
        gblk = np.where(cb < ng1, 2 * cb, 2 * (cb - ng1) + 1)
        jj = np.minimum(gblk * 128 + pos_col, n - 1)
        eip = eids[i][jj].astype(np.int64)
        eip_lo = (eip & 0xFFFF).astype(np.uint16)
        eip_hi = (eip >> 16).astype(np.uint8)
        selm = np.zeros((128, 4), np.float32)
        selm[:, i] = 1.0
        basep = (h * nperm + np.arange(128)).astype(np.float32)[:, None]
        wmat = np.concatenate([
            (np.asarray(W_rel[i, 0], np.float32) * e_scale),
            (np.asarray(W_root[i, 0], np.float32) * e_scale),
            np.asarray(W_rel[i, 1], np.float32),
            np.asarray(W_root[i, 1], np.float32),
            np.tile(query_scaled[i], (128, 1)).astype(np.float32),
            ident,
        ], axis=1).astype(bf16)
        in_maps.append(dict(
            e_sh=np.ascontiguousarray(e_q[c * nsh:(c + 1) * nsh]),
            g_lo=np.concatenate([lo16, eip_lo], axis=1),
            g_u8=np.concatenate([pk8, dl8, eip_hi], axis=1),
            cst=np.concatenate([iota, selm, basep],
                               axis=1).astype(np.float32),
            wts=wmat,
        ))

    nc = build_program(etab, ng1, nb1, ng2)
    nc.compile()
    kernel.last_nc = nc
    kernel.last_in_maps = in_maps
    res = run_bass_kernel_spmd(nc, in_maps, core_ids=list(range(N_CORES)),
                               trace=trace)

    # interleave parity sides back to node order
    a_rows = np.concatenate(
        [res.results[c]["out_part"].astype(np.float32) for c in (0, 2, 4, 6)],
        axis=0).reshape(ng2, 128, 128)
    b_rows = np.concatenate(
        [res.results[c]["out_part"].astype(np.float32) for c in (1, 3, 5, 7)],
        axis=0).reshape(ng2, 128, 128)
    full = np.empty((2 * ng2, 128, 128), np.float32)
    full[0::2] = a_rows
    full[1::2] = b_rows
    out = full.reshape(-1, 128)[:nreg].astype(np.float32)
    kernel.last_results = res
    return out
